# revision 10
# baseline (speedup 1.0000x reference)
"""GroupedTernaryLinear Trainium2 kernel (Bass/Tile, 8-core SPMD), v3.

Computation (matches the jax reference):
  x:      [2, 4096, 4096] f32   -> flatten to [8192, 4096] tokens
  weight: [4096, 1024]    f32
  1. xn = rms_norm(x) over last dim (eps = f32 eps)
  2. w_bf = bf16(weight); per flat 64-chunk: scale = bf16(clip(mean|w_bf|)),
     q = clip(round(w_bf/scale), -1, 1)  ->  wq = q*scale  (exact in bf16)
  3. out[t, g*1024+o] = sum_i xn[t, g*1024+i] * wq[g*1024+o, i]   (4 groups)

v3 layout/pipeline:
  - Host passes bf16-cast, pre-transposed layouts (pure layout/dtype prep):
      xb [1024, 4096] token-major, xt [4096, 1024] feature-major,
      wt [1024, 4096] transposed weight, plus tiny 64-chunk selector consts.
  - Weight quantized per group-stripe g on-chip, software-pipelined so
    stripe g+1 quantizes (ACT/DVE/GpSimd + a few selector matmuls) while
    stripe g's main matmuls run on PE.  Within each token-block slot the
    main matmuls are emitted FIRST so the in-order PE queue never waits on
    quant dependencies.
  - Quant math: |w| via ACT Abs; 64-chunk means via selector-matmul (f32
    psum); thr=THR*s_bf16 broadcast back via hi/lo selector-matmuls; ACT
    evacuates thr (bf16) and s=thr/THR (bf16, exact); q = 2*(w>thr)-(|w|>thr)
    with the w-compare on GpSimd; wq = q*s.
  - rms stats on DVE (tensor_tensor_reduce), fac folded into psum evac
    (ACT Copy(scale=fac) / DVE tensor_scalar_mul alternating).
"""

import os
import sys

sys.path.insert(0, "/opt/trn_rl_repo")

import numpy as np
import ml_dtypes

import concourse.bass as bass
import concourse.mybir as mybir
import concourse.tile as tile
from concourse import bacc
from concourse.bass_utils import run_bass_kernel_spmd

F32 = mybir.dt.float32
BF16 = mybir.dt.bfloat16
AF = mybir.ActivationFunctionType
ALU = mybir.AluOpType
BF = ml_dtypes.bfloat16

N_CORES = 8
T = 1024          # tokens per core
D = 4096          # feature dim (= 4 groups * 1024)
G = 4             # groups
GI = 1024         # group input dim
GO = 1024         # group output dim
GK = GI // 128    # 8 k-chunks per group
TB = T // 128     # 8 token blocks per core
EPS = 1.1920929e-07          # np.finfo(np.float32).eps
THR = 0.5009765625           # bf16 round-to-nearest-even threshold for |r|>0.5

LAST_EXEC_NS = None
LAST_RESULTS = None


def _build():
    nc = bacc.Bacc("TRN2", target_bir_lowering=False, debug=False)
    xb_ap = nc.dram_tensor("xb", [T, D], BF16, kind="ExternalInput").ap()
    xt_ap = nc.dram_tensor("xt", [D, T], BF16, kind="ExternalInput").ap()
    wt_ap = nc.dram_tensor("wt", [GI, D], BF16, kind="ExternalInput").ap()
    sel_ap = nc.dram_tensor("sel", [128, GK, 16], BF16, kind="ExternalInput").ap()
    bsel_ap = nc.dram_tensor("bsel", [16, GK, 128], BF16, kind="ExternalInput").ap()
    out_ap = nc.dram_tensor("out", [T, D], F32, kind="ExternalOutput").ap()
    scr_ap = nc.dram_tensor("scr", [16, 512], F32, kind="ExternalOutput").ap()

    with tile.TileContext(nc) as tc:
        _body(tc, nc, out_ap, xb_ap, xt_ap, wt_ap, sel_ap, bsel_ap, scr_ap)

    nc.compile()
    return nc


def _body(tc, nc, out_ap, xb_ap, xt_ap, wt_ap, sel_ap, bsel_ap, scr_ap):
    with (
        tc.tile_pool(name="consts", bufs=1) as consts,
        tc.tile_pool(name="wtg", bufs=2) as wtg_pool,
        tc.tile_pool(name="xtg", bufs=2) as xtg_pool,
        tc.tile_pool(name="absg", bufs=1) as abs_pool,
        tc.tile_pool(name="wqg", bufs=2) as wq_pool,
        tc.tile_pool(name="smalls", bufs=2) as small_pool,
        tc.tile_pool(name="thrsb", bufs=4) as thr_pool,
        tc.tile_pool(name="qtmp", bufs=3) as qtmp_pool,
        tc.tile_pool(name="xbin", bufs=2) as xb_pool,
        tc.tile_pool(name="stats", bufs=2) as stats_pool,
        tc.tile_pool(name="outsb", bufs=3) as out_pool,
        tc.tile_pool(name="ps_mm", bufs=2, space="PSUM") as ps_mm,
        tc.tile_pool(name="ps_thr", bufs=2, space="PSUM") as ps_thr,
        tc.tile_pool(name="ps_s", bufs=1, space="PSUM") as ps_s,
    ):
        sel = consts.tile([128, GK, 16], BF16, name="sel")
        nc.sync.dma_start(sel[:], sel_ap[:, :, :])
        bsel = consts.tile([16, GK, 128], BF16, name="bsel")
        nc.sync.dma_start(bsel[:], bsel_ap[:, :, :])
        eps_t = consts.tile([128, 1], F32, name="eps_t")
        nc.vector.memset(eps_t[:], EPS)
        fac_all = consts.tile([128, TB], F32, name="fac_all")
        junk = consts.tile([128, D], BF16, name="junk")

        wts = [None] * G
        xts = [None] * G
        wqs = [None] * G

        def dma_stripe(g):
            wts[g] = wtg_pool.tile([128, GK, GI], BF16, name="wt_g")
            xts[g] = xtg_pool.tile([128, GK, T], BF16, name="xt_g")
            gsl = slice(g * GO, (g + 1) * GO)
            for j in range(GK):
                nc.sync.dma_start(
                    wts[g][:, j, :], wt_ap[j * 128:(j + 1) * 128, gsl])
                nc.sync.dma_start(
                    xts[g][:, j, :],
                    xt_ap[g * GI + j * 128:g * GI + (j + 1) * 128, :])

        dma_stripe(0)
        xbts = []
        for tb in range(TB):
            xbt = xb_pool.tile([128, D], BF16, name="xbt")
            nc.sync.dma_start(xbt[:], xb_ap[tb * 128:(tb + 1) * 128, :])
            xbts.append(xbt)

        # Warmup burst keeps PE busy through the prologue so HAM unthrottles;
        # result goes to a scratch output (prevents DCE).
        wu = ps_s.tile([16, 512], F32, name="wu")
        for i in range(32):
            nc.tensor.matmul(wu[:], xts[0][:, 0, 0:16], xts[0][:, 0, 0:512],
                             start=(i == 0), stop=(i == 31))
        wusb = consts.tile([16, 512], F32, name="wusb")
        nc.vector.tensor_copy(wusb[:], wu[:])
        nc.gpsimd.dma_start(scr_ap[:, :], wusb[:])

        def stats_tb(tb):
            ss = stats_pool.tile([128, 1], F32, name="ss")
            nc.scalar.activation(junk[:], xbts[tb][:], AF.Square,
                                 accum_out=ss[:])
            sq = stats_pool.tile([128, 1], F32, name="sq")
            nc.scalar.activation(sq[:], ss[:], AF.Sqrt, bias=eps_t[:],
                                 scale=1.0 / D)
            nc.vector.reciprocal(fac_all[:, tb:tb + 1], sq[:])

        def quant_abs(g):
            ab = abs_pool.tile([128, GK, GI], BF16, name="absg")
            for j in range(GK):
                nc.scalar.activation(ab[:, j, :], wts[g][:, j, :], AF.Abs)
            return ab

        def quant_scale(g, ab):
            sm = small_pool.tile([16, GI], BF16, name="sm")
            for oc in range(2):
                pss = ps_s.tile([16, 512], F32, name="pss")
                for j in range(GK):
                    nc.tensor.matmul(
                        pss[:], sel[:, j, :], ab[:, j, oc * 512:(oc + 1) * 512],
                        start=(j == 0), stop=(j == GK - 1))
                osl = slice(oc * 512, (oc + 1) * 512)
                nc.vector.tensor_scalar(sm[:, osl], pss[:], 1e-8, None, ALU.max)
            wqs[g] = wq_pool.tile([128, GK, GO], BF16, name="wq_g")
            return sm

        def quant_chunk(g, ab, sm, j, oc):
            osl = slice(oc * 512, (oc + 1) * 512)
            psb = ps_thr.tile([128, 512], F32, name="psb")
            nc.tensor.matmul(psb[:], bsel[:, j, :], sm[:, osl],
                             start=True, stop=True)
            sbb = thr_pool.tile([128, 512], BF16, name="sbb")
            nc.scalar.activation(sbb[:], psb[:], AF.Copy)
            mask = qtmp_pool.tile([128, 512], BF16, name="mask")
            nc.vector.scalar_tensor_tensor(mask[:], ab[:, j, osl], 1.0 / THR,
                                           sbb[:], ALU.mult, ALU.is_gt)
            mp = qtmp_pool.tile([128, 512], BF16, name="mp")
            nc.vector.scalar_tensor_tensor(mp[:], wts[g][:, j, osl], 1.0 / THR,
                                           sbb[:], ALU.mult, ALU.is_gt)
            q = qtmp_pool.tile([128, 512], BF16, name="q")
            nc.vector.scalar_tensor_tensor(q[:], mp[:], 2.0, mask[:],
                                           ALU.mult, ALU.subtract)
            nc.vector.tensor_tensor(wqs[g][:, j, osl], q[:], sbb[:], ALU.mult)

        # ---- prologue: quantize stripe 0 fully; stats for first blocks ----
        ab0 = quant_abs(0)
        sm0 = quant_scale(0, ab0)
        for j in range(GK):
            for oc in range(2):
                quant_chunk(0, ab0, sm0, j, oc)
        for tb in range(TB):
            stats_tb(tb)

        qstate = {}

        for g in range(G):
            for tb in range(TB):
                # main matmuls first in each slot (PE in-order)
                pm0 = ps_mm.tile([128, 512], F32, name="pm0")
                pm1 = ps_mm.tile([128, 512], F32, name="pm1")
                tsl = slice(tb * 128, (tb + 1) * 128)
                for k in range(GK):
                    lhsT = xts[g][:, k, tsl]
                    nc.tensor.matmul(pm0[:], lhsT, wqs[g][:, k, 0:512],
                                     start=(k == 0), stop=(k == GK - 1))
                    nc.tensor.matmul(pm1[:], lhsT, wqs[g][:, k, 512:1024],
                                     start=(k == 0), stop=(k == GK - 1))
                fac = fac_all[:, tb:tb + 1]
                ob = out_pool.tile([128, GO], F32, name="ob")
                if g > 0 and tb % 2 == 1:
                    nc.scalar.activation(ob[:, 0:512], pm0[:], AF.Copy, scale=fac)
                    nc.scalar.activation(ob[:, 512:1024], pm1[:], AF.Copy,
                                         scale=fac)
                else:
                    nc.vector.tensor_scalar_mul(ob[:, 0:512], pm0[:], fac)
                    nc.vector.tensor_scalar_mul(ob[:, 512:1024], pm1[:], fac)
                nc.gpsimd.dma_start(out_ap[tsl, g * GO:(g + 1) * GO], ob[:])

                # interleaved next-stripe work after the slot's matmuls
                if g + 1 < G:
                    if tb == 0:
                        dma_stripe(g + 1)
                        qstate['ab'] = quant_abs(g + 1)
                    elif tb == 1:
                        qstate['sm'] = quant_scale(g + 1, qstate['ab'])
                    elif 2 <= tb <= 5:
                        for c in range(4 * (tb - 2), 4 * (tb - 2) + 4):
                            quant_chunk(g + 1, qstate['ab'], qstate['sm'],
                                        c // 2, c % 2)


_NC_CACHE = None
_SEL_CACHE = None


def _make_selectors():
    global _SEL_CACHE
    if _SEL_CACHE is None:
        sel = np.zeros((128, GK, 16), dtype=BF)
        bsel = np.zeros((16, GK, 128), dtype=BF)
        for j in range(GK):
            sel[0:64, j, 2 * j] = BF(1.0 / 64.0)
            sel[64:128, j, 2 * j + 1] = BF(1.0 / 64.0)
            bsel[2 * j, j, 0:64] = BF(1.0)
            bsel[2 * j + 1, j, 64:128] = BF(1.0)
        _SEL_CACHE = (sel, bsel)
    return _SEL_CACHE


def _ensure_ntff_hook():
    """Install the antenv.axon_hooks shim + ctypes NTFF hook if missing."""
    import types

    try:
        from antenv.axon_hooks import get_axon_ntff_profile_hook  # noqa: F401
        return
    except ImportError:
        pass
    import antenv

    mod = types.ModuleType("antenv.axon_hooks")
    mod._hook = None
    mod.set_axon_ntff_profile_hook = lambda h: setattr(mod, "_hook", h)
    mod.get_axon_ntff_profile_hook = lambda: mod._hook
    sys.modules["antenv.axon_hooks"] = mod
    antenv.axon_hooks = mod
    try:
        if "/root/.axon_site" not in sys.path:
            sys.path.insert(0, "/root/.axon_site")
        from trn_agent_boot.trn_boot import _ntff_profile_via_ctypes

        mod.set_axon_ntff_profile_hook(
            _ntff_profile_via_ctypes("/opt/axon/libaxon_pjrt.so")
        )
    except Exception:
        pass


def kernel(x: np.ndarray, weight: np.ndarray) -> np.ndarray:
    global LAST_EXEC_NS, LAST_RESULTS, _NC_CACHE
    x = np.ascontiguousarray(np.asarray(x, dtype=np.float32))
    weight = np.ascontiguousarray(np.asarray(weight, dtype=np.float32))
    lead = x.shape[:-1]
    xf = x.reshape(-1, D)
    assert xf.shape[0] == N_CORES * T, xf.shape

    if _NC_CACHE is None:
        _NC_CACHE = _build()
    nc = _NC_CACHE

    sel, bsel = _make_selectors()
    wt = np.ascontiguousarray(weight.astype(BF).T)          # [1024, 4096] bf16
    xb_all = xf.astype(BF)                                  # [8192, 4096] bf16
    in_maps = []
    for i in range(N_CORES):
        xbc = xb_all[i * T:(i + 1) * T]
        in_maps.append({
            "xb": xbc,
            "xt": np.ascontiguousarray(xbc.T),
            "wt": wt,
            "sel": sel,
            "bsel": bsel,
        })
    trace = bool(int(os.environ.get("CCK_TRACE", "0")))
    kw = {}
    if trace:
        _ensure_ntff_hook()
        tdir = os.environ.get("CCK_TRACE_DIR")
        if tdir:
            os.makedirs(tdir, exist_ok=True)
            kw["tmpdir"] = tdir
    res = run_bass_kernel_spmd(nc, in_maps, list(range(N_CORES)), trace=trace, **kw)
    LAST_EXEC_NS = res.exec_time_ns
    LAST_RESULTS = res
    out = np.concatenate([res.results[i]["out"] for i in range(N_CORES)], axis=0)
    return out.reshape(*lead, D).astype(np.float32, copy=False)


if __name__ == "__main__":
    rng = np.random.default_rng(0)
    x = rng.standard_normal((2, 4096, 4096), dtype=np.float32)
    w = (rng.standard_normal((4096, 1024), dtype=np.float32) * 0.02).astype(np.float32)
    o = kernel(x, w)
    print(o.shape, o.dtype, LAST_EXEC_NS)
